# revision 1
# baseline (speedup 1.0000x reference)
"""GRU Trainium kernel builder + host-side data packing.

Per-core problem: B=32, T steps, H=512, 2 layers, gates [r,z,n].
Layout decisions (see design notes):
  - matmul option A: out[b, h] in psum, lhsT = h^T (bf16), rhs = W (bf16).
  - col-tiling: 4 strips x 32 partitions; units:
      bankA strips: s0=r0, s1=r1, s2=z0, s3=z1
      bankB strips: s0=ni0, s1=ni1, s2=nh0, s3=nh1
  - gate math packed over both layers: [64, 512] ops; SBUF intermediates:
      rz_sb [128,512] (sigma out: parts 0:64 = r(l0,l1), 64:128 = z(l0,l1))
      tmp/u/n at parts 0:64; v at parts 64:128; w at 0:64; h' at 0:64.
  - h state: h_sb [64,512] fp32 (parts 0:32 = h0, 32:64 = h1), ping-pong.
  - h^T state: per layer ring of [128, 128] bf16 tiles (4 k-chunks x 32 batch).
  - L1 runs SKEW steps behind L0.
"""
import numpy as np
import ml_dtypes
import concourse.bass as bass
from concourse import bacc
import concourse.tile as tile
import concourse.mybir as mybir

F32 = mybir.dt.float32
BF16 = mybir.dt.bfloat16
AF = mybir.ActivationFunctionType

H = 512
BL = 32          # batch per core
NK = 4           # k-chunks of 128
RING = 4         # hT ring depth


def build_gru(T=512, skew=2, n_cores=8, pool_ops=True):
    """Returns compiled Bacc module. DRAM tensor names/shapes:
      inputs: xw [2, T*32] bf16, wh0/wh1/wi1 [128, 6144] bf16,
              wi0a [2, 1536] bf16, biasv [1, 2562] bf16, wfc [128, 8] bf16
      output: out [32, 2] f32
    """
    nc = bacc.Bacc("TRN2", target_bir_lowering=False, debug=False,
                   num_devices=n_cores)
    xw_d = nc.dram_tensor("xw", (2, T * BL), BF16, kind="ExternalInput").ap()
    wh0_d = nc.dram_tensor("wh0", (128, 12 * H), BF16, kind="ExternalInput").ap()
    wh1_d = nc.dram_tensor("wh1", (128, 12 * H), BF16, kind="ExternalInput").ap()
    wi1_d = nc.dram_tensor("wi1", (128, 12 * H), BF16, kind="ExternalInput").ap()
    wi0a_d = nc.dram_tensor("wi0a", (2, 3 * H), BF16, kind="ExternalInput").ap()
    bias_d = nc.dram_tensor("biasv", (1, 5 * H + 2), BF16, kind="ExternalInput").ap()
    wfc_d = nc.dram_tensor("wfc", (128, 8), BF16, kind="ExternalInput").ap()
    out_d = nc.dram_tensor("out", (BL, 2), F32, kind="ExternalOutput").ap()

    with tile.TileContext(nc) as tc:
        import contextlib
        with contextlib.ExitStack() as ctx:
            const = ctx.enter_context(tc.tile_pool(name="const", bufs=1))
            state = ctx.enter_context(tc.tile_pool(name="state", bufs=1))
            scratch = ctx.enter_context(tc.tile_pool(name="scratch", bufs=2))
            pspool = ctx.enter_context(tc.tile_pool(name="ps", bufs=2, space="PSUM"))
            psfc = ctx.enter_context(tc.tile_pool(name="psfc", bufs=1, space="PSUM"))

            # ---- persistent tiles ----
            xw = const.tile([2, T * BL], BF16)
            wh0 = const.tile([128, 12 * H], BF16)
            wh1 = const.tile([128, 12 * H], BF16)
            wi1 = const.tile([128, 12 * H], BF16)
            wi0a = const.tile([2, 3 * H], BF16)
            biasv = const.tile([1, 5 * H + 2], BF16)
            wfc = const.tile([128, 8], BF16)
            for t_, d_ in [(xw, xw_d), (wh0, wh0_d), (wh1, wh1_d), (wi1, wi1_d),
                           (wi0a, wi0a_d), (biasv, bias_d), (wfc, wfc_d)]:
                nc.sync.dma_start(out=t_[:], in_=d_)

            id4 = const.tile([128, 32], F32)
            from concourse.masks import make_identity
            for j_ in range(4):
                make_identity(nc, id4[32 * j_:32 * (j_ + 1), :])

            # h state ping-pong [64, 512] fp32 and hT rings [128,128] bf16
            h_sb = [state.tile([64, H], F32, name=f"h{i}", tag=f"h{i}") for i in range(2)]
            h0T = [state.tile([128, NK * BL], BF16, name=f"h0T{i}", tag=f"h0T{i}") for i in range(RING)]
            h1T = [state.tile([128, NK * BL], BF16, name=f"h1T{i}", tag=f"h1T{i}") for i in range(RING)]
            for t_ in h_sb + h0T + h1T:
                nc.vector.memset(t_[:], 0.0)

            # weight slices helper: w tile, gate g, kchunk c -> [128, 512] rhs
            def wsl(w, g, c):
                return w[:, (3 * c + g) * H:(3 * c + g + 1) * H]

            ones_t = const.tile([1, BL], BF16)
            nc.vector.memset(ones_t[:], 1.0)
            ones_lhs = ones_t[0:1, 0:BL]  # [1, 32] of ones at partition 0

            def bias_rhs(idx):  # idx in {0..4}: bh0n, br1, bz1, bi1n, bh1n
                return biasv[0:1, idx * H:(idx + 1) * H]

            # ---------------- superstep loop ----------------
            n_super = T + skew
            for s in range(n_super):
                l0 = s < T
                l1 = s >= skew
                t0 = s           # L0 timestep
                t1 = s - skew    # L1 timestep
                par = s % 2

                bankA = pspool.tile([128, H], F32, tag="bankA")
                bankB = pspool.tile([128, H], F32, tag="bankB")
                trps = pspool.tile([128, 2 * NK * BL], F32, tag="trps")

                # hT operands (written at end of superstep st for timestep st)
                h0T_l0 = h0T[(t0 - 1) % RING] if t0 >= 1 else None   # h0(t0-1)
                h0T_l1 = h0T[t1 % RING] if l1 else None              # h0(t1)
                h1T_l1 = h1T[(t1 - 1) % RING] if t1 >= 1 else None   # h1(t1-1)

                # ---- build unit MM lists: (psum_slice, strip, [(lhsT, rhs)...]) ----
                # order within unit: independent (gi/bias) first, hid last
                units = []  # list of (out_ap, tile_col, mms)
                if l0:
                    # r0 @ bankA s0 ; z0 @ bankA s2 ; ni0 @ bankB s0 ; nh0 @ bankB s2
                    xt = xw[0:2, BL * t0: BL * (t0 + 1)]  # K=2 lhsT (x_t, ones)
                    r0 = [(xt, wi0a[0:2, 0:H])]
                    z0 = [(xt, wi0a[0:2, H:2 * H])]
                    ni0 = [(xt, wi0a[0:2, 2 * H:3 * H])]
                    nh0 = [(ones_lhs, bias_rhs(0))]
                    if t0 >= 1:
                        for c in range(NK):
                            lh = h0T_l0[:, BL * c: BL * (c + 1)]
                            r0.append((lh, wsl(wh0, 0, c)))
                            z0.append((lh, wsl(wh0, 1, c)))
                            nh0.append((lh, wsl(wh0, 2, c)))
                    units += [(bankA[0:32, :], 0, r0), (bankA[64:96, :], 64, z0),
                              (bankB[0:32, :], 0, ni0), (bankB[64:96, :], 64, nh0)]
                if l1:
                    r1 = [(ones_lhs, bias_rhs(1))]
                    z1 = [(ones_lhs, bias_rhs(2))]
                    ni1 = [(ones_lhs, bias_rhs(3))]
                    nh1 = [(ones_lhs, bias_rhs(4))]
                    for c in range(NK):
                        lh = h0T_l1[:, BL * c: BL * (c + 1)]
                        r1.append((lh, wsl(wi1, 0, c)))
                        z1.append((lh, wsl(wi1, 1, c)))
                        ni1.append((lh, wsl(wi1, 2, c)))
                    if t1 >= 1:
                        for c in range(NK):
                            lh = h1T_l1[:, BL * c: BL * (c + 1)]
                            r1.append((lh, wsl(wh1, 0, c)))
                            z1.append((lh, wsl(wh1, 1, c)))
                            nh1.append((lh, wsl(wh1, 2, c)))
                    units += [(bankA[32:64, :], 32, r1), (bankA[96:128, :], 96, z1),
                              (bankB[32:64, :], 32, ni1), (bankB[96:128, :], 96, nh1)]

                # ---- emit MMs round-robin across strips ----
                by_strip = {}
                for out_ap, col, mms in units:
                    by_strip.setdefault(col, []).append((out_ap, mms, [False]))
                # flatten: per strip, a queue of (out_ap, mm, is_first, is_last)
                queues = {}
                for col, us in by_strip.items():
                    qi, qd = [], []
                    for out_ap, mms, _ in us:
                        n_indep = len(mms) - (NK if (mms and mms[-1][1].tensor.name.startswith(("wh0", "wh1"))) else 0)
                        for i, mm in enumerate(mms):
                            ent = (out_ap, mm, i == 0, i == len(mms) - 1)
                            (qi if i < n_indep else qd).append(ent)
                    queues[col] = qi + qd
                maxlen = max(len(q) for q in queues.values())
                for i in range(maxlen):
                    for col in sorted(queues):
                        q = queues[col]
                        if i < len(q):
                            out_ap, (lh, rh), first, last = q[i]
                            nc.tensor.matmul(out_ap, lhsT=lh, rhs=rh,
                                             start=first, stop=last,
                                             tile_position=(0, col),
                                             skip_group_check=True)

                # ---- gate math ----
                # active partition windows
                if l0 and l1:
                    lo, hi = 0, 64
                elif l0:
                    lo, hi = 0, 32
                else:
                    lo, hi = 32, 64
                n_act = hi - lo

                rz = scratch.tile([128, H], F32, tag="rz")
                tmp = scratch.tile([64, H], F32, tag="tmp")
                u = scratch.tile([64, H], F32, tag="u")
                nn_ = scratch.tile([64, H], F32, tag="nn")
                vw = scratch.tile([128, H], F32, tag="vw")
                hnew = h_sb[par]
                hold = h_sb[1 - par]

                # sigma over r and z regions of bankA (restrict to active windows)
                nc.scalar.activation(rz[lo:hi, :], bankA[lo:hi, :], AF.Sigmoid)
                nc.scalar.activation(rz[64 + lo:64 + hi, :], bankA[64 + lo:64 + hi, :], AF.Sigmoid)
                # tmp = r * nh   (SBUF x PSUM, windows may differ)
                nc.vector.tensor_mul(out=tmp[lo:hi, :], in0=rz[lo:hi, :], in1=bankB[64 + lo:64 + hi, :])
                # u = tmp + ni
                nc.vector.tensor_add(out=u[lo:hi, :], in0=tmp[lo:hi, :], in1=bankB[lo:hi, :])
                # n = tanh(u)
                nc.scalar.activation(nn_[lo:hi, :], u[lo:hi, :], AF.Tanh)
                # v = h_old - n   -> parts 64+
                eng_a = nc.gpsimd if pool_ops else nc.vector
                eng_b = nc.gpsimd if pool_ops else nc.vector
                eng_a.tensor_sub(out=vw[64 + lo:64 + hi, :], in0=hold[lo:hi, :], in1=nn_[lo:hi, :])
                # w = z * v -> parts 0:64 of vw
                eng_b.tensor_mul(out=vw[lo:hi, :], in0=rz[64 + lo:64 + hi, :], in1=vw[64 + lo:64 + hi, :])
                # h' = n + w
                nc.vector.tensor_add(out=hnew[lo:hi, :], in0=nn_[lo:hi, :], in1=vw[lo:hi, :])

                # ---- transposes: h'(layer) [32,512] -> hT [128, 128] bf16 ----
                ident = None
                for (active, base, ring, tstep) in [
                    (l0, 0, h0T, t0), (l1, 32, h1T, t1)]:
                    if not active:
                        continue
                    dst = ring[tstep % RING]
                    off = 0 if base == 0 else NK * BL
                    for c in range(NK):
                        nc.tensor.transpose(
                            trps[:, off + BL * c: off + BL * (c + 1)],
                            hnew[base:base + 32, 128 * c:128 * (c + 1)],
                            id4[base:base + 32, :],
                            tile_position=(base, 0),
                        )
                    nc.vector.tensor_copy(out=dst[:], in_=trps[:, off:off + NK * BL])

            # ---- FC ----
            ps_fc = psfc.tile([BL, 2], F32)
            hT_last = h1T[(T - 1) % RING]
            for c in range(NK):
                nc.tensor.matmul(ps_fc[:, :], lhsT=hT_last[:, BL * c:BL * (c + 1)],
                                 rhs=wfc[:, 2 * c:2 * (c + 1)],
                                 start=(c == 0), stop=False, skip_group_check=True)
            nc.tensor.matmul(ps_fc[:, :], lhsT=ones_lhs,
                             rhs=biasv[0:1, 5 * H:5 * H + 2],
                             start=False, stop=True, skip_group_check=True)
            out_sb = const.tile([BL, 2], F32)
            nc.vector.tensor_copy(out=out_sb[:], in_=ps_fc[:, :])
            nc.sync.dma_start(out=out_d, in_=out_sb[:])

    nc.compile()
    return nc


# ---------------- host-side packing ----------------

def pack_inputs(x, Wi0, bi0, Wi_rest, bi_rest, Wh, bh, fc_w, fc_b, n_cores=8):
    """Full inputs -> list of per-core in_maps."""
    B, T = x.shape
    bl = B // n_cores
    assert bl == BL

    def w_pack(W3):  # [3, H, H] -> [128, 12*H] with [p, (3c+g)*H + n] = W3[g, n, 128c+p]
        a = W3.transpose(2, 0, 1)            # [i, g, n]
        a = a.reshape(NK, 128, 3, H)         # [c, p, g, n]
        a = a.transpose(1, 0, 2, 3)          # [p, c, g, n]
        return np.ascontiguousarray(a).reshape(128, 12 * H).astype(ml_dtypes.bfloat16)

    wh0 = w_pack(Wh[0]); wh1 = w_pack(Wh[1]); wi1 = w_pack(Wi_rest[0])

    wi0a = np.zeros((2, 3 * H), np.float32)
    for g in range(3):
        wi0a[0, g * H:(g + 1) * H] = Wi0[g, :, 0]
    wi0a[1, 0:H] = bi0[0] + bh[0, 0]
    wi0a[1, H:2 * H] = bi0[1] + bh[0, 1]
    wi0a[1, 2 * H:3 * H] = bi0[2]
    wi0a = wi0a.astype(ml_dtypes.bfloat16)

    biasv = np.zeros((1, 5 * H + 2), np.float32)
    biasv[0, 0:H] = bh[0, 2]
    biasv[0, H:2 * H] = bi_rest[0, 0] + bh[1, 0]
    biasv[0, 2 * H:3 * H] = bi_rest[0, 1] + bh[1, 1]
    biasv[0, 3 * H:4 * H] = bi_rest[0, 2]
    biasv[0, 4 * H:5 * H] = bh[1, 2]
    biasv[0, 5 * H:] = fc_b
    biasv = biasv.astype(ml_dtypes.bfloat16)

    wfc = fc_w.T.reshape(NK, 128, 2).transpose(1, 0, 2)
    wfc = np.ascontiguousarray(wfc).reshape(128, 8).astype(ml_dtypes.bfloat16)

    in_maps = []
    for c in range(n_cores):
        xc = x[c * bl:(c + 1) * bl, :]       # [32, T]
        xw = np.empty((2, T * bl), np.float32)
        xw[0] = xc.T.reshape(-1)             # [t*32 + b]
        xw[1] = 1.0
        in_maps.append({
            "xw": xw.astype(ml_dtypes.bfloat16),
            "wh0": wh0, "wh1": wh1, "wi1": wi1,
            "wi0a": wi0a, "biasv": biasv, "wfc": wfc,
        })
    return in_maps


def unpack_outputs(results):
    return np.concatenate([r["out"] for r in results], axis=0)


# ---------------- public entry point ----------------
_CACHED = {}

def _get_nc(T):
    if T not in _CACHED:
        _CACHED[T] = build_gru(T=T)
    return _CACHED[T]


def kernel(x, Wi0, bi0, Wi_rest, bi_rest, Wh, bh, fc_w, fc_b):
    """Full-input 2-layer GRU (B=256, H=512) on 8 NeuronCores.

    Sharding: data-parallel over batch (32 per core), weights replicated.
    Inside: bf16 matmuls (col-tiled option-A layout), fp32 psum/state,
    per-step gate math packed across both layers, PE transposes for h^T.
    """
    from concourse.bass_utils import run_bass_kernel_spmd
    x = np.asarray(x); Wi0 = np.asarray(Wi0); bi0 = np.asarray(bi0)
    Wi_rest = np.asarray(Wi_rest); bi_rest = np.asarray(bi_rest)
    Wh = np.asarray(Wh); bh = np.asarray(bh)
    fc_w = np.asarray(fc_w); fc_b = np.asarray(fc_b)
    T = x.shape[1]
    nc = _get_nc(T)
    in_maps = pack_inputs(x, Wi0, bi0, Wi_rest, bi_rest, Wh, bh, fc_w, fc_b)
    res = run_bass_kernel_spmd(nc, in_maps, core_ids=list(range(8)))
    return unpack_outputs(res.results).astype(np.float32)



# revision 2
# speedup vs baseline: 1.1072x; 1.1072x over previous
"""GRU Trainium kernel v2: fp8 DoubleRow r/z gates + bf16 n-gate.

Per-core: B=32, T steps, H=512, 2 layers, skew=1 (L1 lags L0 by one step).

Design:
  - r/z gate GEMMs: fp8e4 DoubleRow (K=256/MM, 0.5 cyc/row), fused across
    layers via partition-split K: M=64 (rows 0:32=L0, 32:64=L1), base 0
    (DoubleRow ISA requires dst partition 0). 6 fused MMs per gate:
      f=0..3: p<64 -> wh0_g @ h0T(t0-1); p>=64 -> wi1_g @ h0T(t1)  [same tile,
              skew=1 means t1 = t0-1]; block-diagonal lhsT (zeros keep the
              two column groups independent).
      f=4,5:  full-partition wh1_g @ h1T(t1-1); lhsT cols 0:32 zero.
  - n gate stays bf16 (fp8 there breaks the 2e-2 gate; r/z in fp8 adds <1e-3).
  - x-input rank-1 GEMMs + all biases folded into 3 "combo" selector MMs
    (K<=5 bf16), one per psum bank.
  - gate math packed [64, 512] over both layers, n-path split into 256-wide
    halves; bf16 intermediates in SBUF (DVE 2x/4x modes); h' = oz*n + z*h
    with oz = sigmoid(-z_pre) (one stage shorter than (h-n)*z+n).
  - h state bf16; PE transposes produce bf16 hT rings: classic chunk rings
    (bf16 for n-gate, fp8 for wh1-side) + a half-layout fp8 ring for the
    fused f=0..3 lhsT (64-partition halves, block-diagonal with zero cols).
"""
import numpy as np
import ml_dtypes
import concourse.bass as bass
from concourse import bacc
import concourse.tile as tile
import concourse.mybir as mybir

F32 = mybir.dt.float32
BF16 = mybir.dt.bfloat16
F8 = mybir.dt.float8e4
NF8 = ml_dtypes.float8_e4m3
NBF = ml_dtypes.bfloat16
AF = mybir.ActivationFunctionType
DR = mybir.MatmulPerfMode.DoubleRow

H = 512
BL = 32          # batch per core
RING = 4


def build_gru(T=512, n_cores=8):
    nc = bacc.Bacc("TRN2", target_bir_lowering=False, debug=False,
                   num_devices=n_cores)
    xw_d = nc.dram_tensor("xw", (1, T * BL), BF16, kind="ExternalInput").ap()
    wn_d = nc.dram_tensor("wn", (128, 12 * H), BF16, kind="ExternalInput").ap()
    wrz_d = nc.dram_tensor("wrz8", (128, 24 * H), F8, kind="ExternalInput").ap()
    cA_d = nc.dram_tensor("cA", (3, H), BF16, kind="ExternalInput").ap()
    cB_d = nc.dram_tensor("cB", (3, H), BF16, kind="ExternalInput").ap()
    cC_d = nc.dram_tensor("cC", (5, H), BF16, kind="ExternalInput").ap()
    wfc_d = nc.dram_tensor("wfc", (128, 8), BF16, kind="ExternalInput").ap()
    fcb_d = nc.dram_tensor("fcb", (1, 2), BF16, kind="ExternalInput").ap()
    xsl_d = nc.dram_tensor("xsl", (5, 128), BF16, kind="ExternalInput").ap()
    out_d = nc.dram_tensor("out", (BL, 2), F32, kind="ExternalOutput").ap()

    with tile.TileContext(nc) as tc:
        import contextlib
        with contextlib.ExitStack() as ctx:
            const = ctx.enter_context(tc.tile_pool(name="const", bufs=1))
            state = ctx.enter_context(tc.tile_pool(name="state", bufs=1))
            scratch = ctx.enter_context(tc.tile_pool(name="scratch", bufs=2))
            psR = ctx.enter_context(tc.tile_pool(name="psR", bufs=2, space="PSUM"))
            psZ = ctx.enter_context(tc.tile_pool(name="psZ", bufs=2, space="PSUM"))
            psN = ctx.enter_context(tc.tile_pool(name="psN", bufs=2, space="PSUM"))
            psT = ctx.enter_context(tc.tile_pool(name="psT", bufs=1, space="PSUM"))
            psF = ctx.enter_context(tc.tile_pool(name="psF", bufs=1, space="PSUM"))

            # ---- persistent inputs ----
            xw = const.tile([1, T * BL], BF16)
            wn = const.tile([128, 12 * H], BF16)
            wrz = const.tile([128, 24 * H], F8)
            cA = const.tile([3, H], BF16)
            cB = const.tile([3, H], BF16)
            cC = const.tile([5, H], BF16)
            wfc = const.tile([128, 8], BF16)
            fcb = const.tile([1, 2], BF16)
            for t_, d_ in [(xw, xw_d), (wn, wn_d), (wrz, wrz_d), (cA, cA_d),
                           (cB, cB_d), (cC, cC_d), (wfc, wfc_d), (fcb, fcb_d)]:
                nc.sync.dma_start(out=t_[:], in_=d_)

            from concourse.masks import make_identity
            id2 = const.tile([64, 32], BF16)
            make_identity(nc, id2[0:32, :])
            make_identity(nc, id2[32:64, :])
            ones_t = const.tile([1, BL], BF16)
            nc.vector.memset(ones_t[:], 1.0)

            # xsel: [5,128] rows: x@0:32 | 1@0:32 | 1@32:64 | 1@64:96 | 1@96:128
            xsel = [state.tile([5, 128], BF16, name=f"xs{i}") for i in range(2)]
            for xs in xsel:
                nc.sync.dma_start(out=xs[:], in_=xsl_d)

            # h state ping-pong [64, 512] bf16 (rows 0:32 L0, 32:64 L1)
            h_sb = [state.tile([64, H], BF16, name=f"h{i}") for i in range(2)]
            # rings
            ra8 = [state.tile([128, 512], F8, name=f"ra8_{i}") for i in range(RING)]
            rb8 = [state.tile([128, 256], F8, name=f"rb8_{i}") for i in range(RING)]
            rb0 = [state.tile([128, 128], BF16, name=f"rb0_{i}") for i in range(RING)]
            rb1 = [state.tile([128, 128], BF16, name=f"rb1_{i}") for i in range(RING)]
            for t_ in h_sb + ra8 + rb8 + rb0 + rb1:
                nc.vector.memset(t_[:], 0.0)

            def dr_lhsT_a(ring_t, f):       # fused lhsT f=0..3: [128, 2, 64]
                return ring_t[:, 128 * f:128 * (f + 1)].rearrange(
                    "p (k m) -> p k m", k=2)

            def dr_lhsT_b(ring_t, fp):      # fused lhsT f=4,5 -> f'=0,1
                return ring_t[:, 128 * fp:128 * (fp + 1)].rearrange(
                    "p (k m) -> p k m", k=2)

            def dr_rhs(g, f):               # [128, 2, 512]
                o = ((g * 6 + f) * 2) * H
                return wrz[:, o:o + 2 * H].rearrange("p (k n) -> p k n", k=2)

            n_super = T + 2       # skew=2: L1 lags L0 by two steps
            pending = [None]        # deferred (hnew, w0, w1, t0, t1) of prev step

            def emit_rings_l1(hnew, tt1):
                """L1 transposes of prev step + rb1/rb8 ring writes."""
                trp = psT.tile([128, 512], BF16, tag="trp")
                trp1 = trp[:, 384:512]
                for c in range(4):
                    nc.tensor.transpose(
                        trp1[:, 32 * c:32 * (c + 1)],
                        hnew[32:64, 128 * c:128 * (c + 1)],
                        id2[32:64, :], tile_position=(32, 0))
                nc.scalar.activation(rb1[tt1 % RING][:], trp1[:, :], AF.Copy)
                nc.vector.tensor_copy(
                    out=rb8[tt1 % RING][:].rearrange("p (c m) -> p c m", c=4)[:, :, 32:64],
                    in_=trp1[:, :].rearrange("p (c m) -> p c m", c=4))
                return trp

            def emit_rings_l0(trp, hnew, tt0):
                """L0 transposes of prev step + ra8/rb0 ring writes.

                h0T(tt0) feeds: ra8[(tt0+1)%R] rows 0:64 (consumed next step,
                critical) and ra8[(tt0+2)%R] rows 64:128 (wi1 side, consumed
                in two steps — off the critical path)."""
                trpb = trp[:, 256:384]
                for c in range(4):
                    nc.tensor.transpose(
                        trpb[:, 32 * c:32 * (c + 1)],
                        hnew[0:32, 128 * c:128 * (c + 1)],
                        id2[0:32, :], tile_position=(0, 0))
                ra_f = ra8[(tt0 + 1) % RING]
                ra_o = ra8[(tt0 + 2) % RING]
                lo4 = lambda ap: ap.rearrange("p (f m) -> p f m", f=4)
                # fused-lhsT layout: col 128f + 64kt + 32sel + b
                nc.vector.tensor_copy(out=lo4(ra_f[0:64, :])[:, :, 0:32],
                                      in_=lo4(trpb[0:64, :]))
                nc.vector.tensor_copy(out=lo4(ra_f[0:64, :])[:, :, 64:96],
                                      in_=lo4(trpb[64:128, :]))
                nc.vector.tensor_copy(out=rb0[tt0 % RING][:], in_=trpb[:, :])
                nc.scalar.activation(lo4(ra_o[64:128, :])[:, :, 32:64],
                                     lo4(trpb[0:64, :]), AF.Copy)
                nc.scalar.activation(lo4(ra_o[64:128, :])[:, :, 96:128],
                                     lo4(trpb[64:128, :]), AF.Copy)

            for s in range(n_super):
                t0, t1 = s, s - 2
                l0 = s < T
                l1 = s >= 2
                par = s % 2
                lo = 0 if l0 else 32
                hi = 64 if l1 else 32

                pr = psR.tile([64, H], F32, tag="pr")
                pz = psZ.tile([64, H], F32, tag="pz")
                pn = psN.tile([128, H], F32, tag="pn")

                xs = xsel[par]
                if l0:
                    nc.scalar.activation(xs[0:1, 0:BL], xw[0:1, BL * t0:BL * (t0 + 1)],
                                         AF.Copy)

                ra_cur = ra8[s % RING]            # 0:64 h0T(s-1) | 64:128 h0T(s-2)
                rb8_prev2 = rb8[(t1 - 1) % RING]
                rb0_fresh = rb0[(t0 - 1) % RING]  # nh0 side
                rb0_old = rb0[t1 % RING]          # ni1 side (h0T(t1), 2 steps old)
                rb1_prev2 = rb1[(t1 - 1) % RING]

                # ---- combos (start accumulation in each bank) ----
                nc.tensor.matmul(pr[:, :], lhsT=xs[0:3, 0:64], rhs=cA[:],
                                 start=True, stop=False, tile_position=(0, 0),
                                 skip_group_check=True)
                nc.tensor.matmul(pz[:, :], lhsT=xs[0:3, 0:64], rhs=cB[:],
                                 start=True, stop=False, tile_position=(0, 0),
                                 skip_group_check=True)
                nc.tensor.matmul(pn[:, :], lhsT=xs[0:5, :], rhs=cC[:],
                                 start=True, stop=False, tile_position=(0, 0),
                                 skip_group_check=True)

                # ---- ni1 (2-step-old dep: pure filler work) ----
                for c in range(4):
                    nc.tensor.matmul(pn[32:64, :],
                                     lhsT=rb0_old[:, 32 * c:32 * (c + 1)],
                                     rhs=wn[:, (4 + c) * H:(5 + c) * H],
                                     start=False, stop=(c == 3),
                                     tile_position=(0, 32), skip_group_check=True)

                # ---- deferred L1 rings of prev step (feeds nh1/f45 below) ----
                trp_prev = None
                if pending[0] is not None and pending[0][2]:
                    trp_prev = emit_rings_l1(pending[0][0], pending[0][4])

                # ---- L1-side MMs: nh1, rz f=4,5 ----
                for c in range(4):
                    nc.tensor.matmul(pn[96:128, :],
                                     lhsT=rb1_prev2[:, 32 * c:32 * (c + 1)],
                                     rhs=wn[:, (8 + c) * H:(9 + c) * H],
                                     start=False, stop=(c == 3),
                                     tile_position=(0, 96), skip_group_check=True)
                for fp in range(2):
                    lh = dr_lhsT_b(rb8_prev2, fp)
                    nc.tensor.matmul(pr[:, :], lhsT=lh, rhs=dr_rhs(0, 4 + fp),
                                     start=False, stop=False, perf_mode=DR,
                                     tile_position=(0, 0), skip_group_check=True)
                    nc.tensor.matmul(pz[:, :], lhsT=lh, rhs=dr_rhs(1, 4 + fp),
                                     start=False, stop=False, perf_mode=DR,
                                     tile_position=(0, 0), skip_group_check=True)

                # ---- deferred L0 rings of prev step (feeds f0..3/nh0) ----
                if pending[0] is not None:
                    if pending[0][1]:
                        if trp_prev is None:
                            trp_prev = psT.tile([128, 512], BF16, tag="trp")
                        emit_rings_l0(trp_prev, pending[0][0], pending[0][3])
                    pending[0] = None

                # ---- fresh-dependency MMs: rz f=0..3, then nh0 ----
                for f in range(4):
                    lh = dr_lhsT_a(ra_cur, f)
                    nc.tensor.matmul(pr[:, :], lhsT=lh, rhs=dr_rhs(0, f),
                                     start=False, stop=(f == 3), perf_mode=DR,
                                     tile_position=(0, 0), skip_group_check=True)
                    nc.tensor.matmul(pz[:, :], lhsT=lh, rhs=dr_rhs(1, f),
                                     start=False, stop=(f == 3), perf_mode=DR,
                                     tile_position=(0, 0), skip_group_check=True)
                for c in range(4):
                    nc.tensor.matmul(pn[64:96, :],
                                     lhsT=rb0_fresh[:, 32 * c:32 * (c + 1)],
                                     rhs=wn[:, c * H:(c + 1) * H],
                                     start=False, stop=(c == 3),
                                     tile_position=(0, 64), skip_group_check=True)

                # ---- gate math ----
                rq = scratch.tile([64, H], BF16, tag="rq")
                zq = scratch.tile([64, H], BF16, tag="zq")
                oz = scratch.tile([64, H], BF16, tag="oz")
                tq = scratch.tile([64, H], BF16, tag="tq")
                uq = scratch.tile([64, H], BF16, tag="uq")
                nq = scratch.tile([64, H], BF16, tag="nq")
                zh = scratch.tile([64, H], BF16, tag="zh")
                pq = scratch.tile([64, H], BF16, tag="pq")
                hnew = h_sb[par]
                hold = h_sb[1 - par]

                HH = H // 2
                for hb in range(2):
                    sl = slice(hb * HH, (hb + 1) * HH)
                    nc.scalar.activation(rq[lo:hi, sl], pr[lo:hi, sl], AF.Sigmoid)
                nc.scalar.activation(zq[lo:hi, :], pz[lo:hi, :], AF.Sigmoid)
                nc.scalar.activation(oz[lo:hi, :], pz[lo:hi, :], AF.Sigmoid,
                                     scale=-1.0)
                # zh = z * h_old (gpsimd, sbuf-only)
                nc.gpsimd.tensor_mul(out=zh[lo:hi, :], in0=zq[lo:hi, :],
                                     in1=hold[lo:hi, :])
                for hb in range(2):
                    sl = slice(hb * HH, (hb + 1) * HH)
                    nc.vector.tensor_mul(out=tq[lo:hi, sl], in0=rq[lo:hi, sl],
                                         in1=pn[64 + lo:64 + hi, sl])
                    nc.vector.tensor_add(out=uq[lo:hi, sl], in0=tq[lo:hi, sl],
                                         in1=pn[lo:hi, sl])
                    nc.scalar.activation(nq[lo:hi, sl], uq[lo:hi, sl], AF.Tanh)
                    nc.vector.tensor_mul(out=pq[lo:hi, sl], in0=oz[lo:hi, sl],
                                         in1=nq[lo:hi, sl])
                    nc.vector.tensor_add(out=hnew[lo:hi, sl], in0=pq[lo:hi, sl],
                                         in1=zh[lo:hi, sl])

                # ---- stash transposes + ring writes for next step's stream ----
                pending[0] = (hnew, l0, l1, t0, t1)

            if pending[0] is not None:
                trp_f = None
                if pending[0][2]:
                    trp_f = emit_rings_l1(pending[0][0], pending[0][4])
                if pending[0][1]:
                    if trp_f is None:
                        trp_f = psT.tile([128, 512], BF16, tag="trp")
                    emit_rings_l0(trp_f, pending[0][0], pending[0][3])
                pending[0] = None

            # ---- FC ----
            ps_fc = psF.tile([BL, 2], F32)
            hT_last = rb1[(T - 1) % RING]
            for c in range(4):
                nc.tensor.matmul(ps_fc[:, :], lhsT=hT_last[:, BL * c:BL * (c + 1)],
                                 rhs=wfc[:, 2 * c:2 * (c + 1)],
                                 start=(c == 0), stop=False, skip_group_check=True)
            nc.tensor.matmul(ps_fc[:, :], lhsT=ones_t[0:1, :], rhs=fcb[:],
                             start=False, stop=True, skip_group_check=True)
            out_sb = const.tile([BL, 2], F32)
            nc.vector.tensor_copy(out=out_sb[:], in_=ps_fc[:, :])
            nc.sync.dma_start(out=out_d, in_=out_sb[:])

    nc.compile()
    return nc


# ---------------- host-side packing ----------------

def pack_inputs(x, Wi0, bi0, Wi_rest, bi_rest, Wh, bh, fc_w, fc_b, n_cores=8):
    B, T = x.shape
    bl = B // n_cores
    assert bl == BL

    # n-gate weights, classic chunk layout: [wh0_n, wi1_n, wh1_n]
    wn = np.zeros((128, 12 * H), np.float32)
    for M, W in enumerate([Wh[0, 2], Wi_rest[0, 2], Wh[1, 2]]):
        for c in range(4):
            # wn[p, (4M+c)*H + n] = W[n, 128c+p]
            wn[:, (4 * M + c) * H:(4 * M + c + 1) * H] = W[:, 128 * c:128 * (c + 1)].T
    wn = wn.astype(NBF)

    # r/z fused fp8 rhs
    wrz = np.zeros((128, 24 * H), np.float32)
    for g in range(2):
        Wh0g, Wi1g, Wh1g = Wh[0, g], Wi_rest[0, g], Wh[1, g]
        for f in range(4):
            for kt in range(2):
                col = ((g * 6 + f) * 2 + kt) * H
                k0 = 128 * f + 64 * kt
                wrz[0:64, col:col + H] = Wh0g[:, k0:k0 + 64].T
                wrz[64:128, col:col + H] = Wi1g[:, k0:k0 + 64].T
        for fp in range(2):
            for kt in range(2):
                col = ((g * 6 + 4 + fp) * 2 + kt) * H
                k0 = 256 * fp + 128 * kt
                wrz[:, col:col + H] = Wh1g[:, k0:k0 + 128].T
    wrz = wrz.astype(NF8)

    # combo rhs
    cA = np.stack([Wi0[0, :, 0], bi0[0] + bh[0, 0], bi_rest[0, 0] + bh[1, 0]])
    cB = np.stack([Wi0[1, :, 0], bi0[1] + bh[0, 1], bi_rest[0, 1] + bh[1, 1]])
    cC = np.stack([Wi0[2, :, 0], bi0[2], bi_rest[0, 2], bh[0, 2], bh[1, 2]])
    cA = cA.astype(NBF); cB = cB.astype(NBF); cC = cC.astype(NBF)

    wfc = fc_w.T.reshape(4, 128, 2).transpose(1, 0, 2)
    wfc = np.ascontiguousarray(wfc).reshape(128, 8).astype(NBF)
    fcb = fc_b.reshape(1, 2).astype(NBF)

    xsl = np.zeros((5, 128), np.float32)
    for j in range(4):
        xsl[j + 1, 32 * j:32 * j + 32] = 1.0
    xsl = xsl.astype(NBF)

    in_maps = []
    for cix in range(n_cores):
        xc = x[cix * bl:(cix + 1) * bl, :]
        xw = xc.T.reshape(1, -1).astype(NBF)     # [1, T*32], t-major
        in_maps.append({
            "xw": xw, "wn": wn, "wrz8": wrz,
            "cA": cA, "cB": cB, "cC": cC, "wfc": wfc, "fcb": fcb, "xsl": xsl,
        })
    return in_maps


def unpack_outputs(results):
    return np.concatenate([r["out"] for r in results], axis=0)


# ---------------- public entry point ----------------
_CACHED = {}


def _get_nc(T):
    if T not in _CACHED:
        _CACHED[T] = build_gru(T=T)
    return _CACHED[T]


def kernel(x, Wi0, bi0, Wi_rest, bi_rest, Wh, bh, fc_w, fc_b):
    """Full-input 2-layer GRU (B=256, H=512) on 8 NeuronCores.

    Sharding: data-parallel over batch (32 per core), weights replicated.
    """
    from concourse.bass_utils import run_bass_kernel_spmd
    x = np.asarray(x); Wi0 = np.asarray(Wi0); bi0 = np.asarray(bi0)
    Wi_rest = np.asarray(Wi_rest); bi_rest = np.asarray(bi_rest)
    Wh = np.asarray(Wh); bh = np.asarray(bh)
    fc_w = np.asarray(fc_w); fc_b = np.asarray(fc_b)
    T = x.shape[1]
    nc = _get_nc(T)
    in_maps = pack_inputs(x, Wi0, bi0, Wi_rest, bi_rest, Wh, bh, fc_w, fc_b)
    res = run_bass_kernel_spmd(nc, in_maps, core_ids=list(range(8)))
    return unpack_outputs(res.results).astype(np.float32)


# revision 3
# speedup vs baseline: 1.1225x; 1.0138x over previous
"""GRU Trainium kernel v2: fp8 DoubleRow r/z gates + bf16 n-gate.

Per-core: B=32, T steps, H=512, 2 layers, skew=1 (L1 lags L0 by one step).

Design:
  - r/z gate GEMMs: fp8e4 DoubleRow (K=256/MM, 0.5 cyc/row), fused across
    layers via partition-split K: M=64 (rows 0:32=L0, 32:64=L1), base 0
    (DoubleRow ISA requires dst partition 0). 6 fused MMs per gate:
      f=0..3: p<64 -> wh0_g @ h0T(t0-1); p>=64 -> wi1_g @ h0T(t1)  [same tile,
              skew=1 means t1 = t0-1]; block-diagonal lhsT (zeros keep the
              two column groups independent).
      f=4,5:  full-partition wh1_g @ h1T(t1-1); lhsT cols 0:32 zero.
  - n gate stays bf16 (fp8 there breaks the 2e-2 gate; r/z in fp8 adds <1e-3).
  - x-input rank-1 GEMMs + all biases folded into 3 "combo" selector MMs
    (K<=5 bf16), one per psum bank.
  - gate math packed [64, 512] over both layers, n-path split into 256-wide
    halves; bf16 intermediates in SBUF (DVE 2x/4x modes); h' = oz*n + z*h
    with oz = sigmoid(-z_pre) (one stage shorter than (h-n)*z+n).
  - h state bf16; PE transposes produce bf16 hT rings: classic chunk rings
    (bf16 for n-gate, fp8 for wh1-side) + a half-layout fp8 ring for the
    fused f=0..3 lhsT (64-partition halves, block-diagonal with zero cols).
"""
import numpy as np
import ml_dtypes
import concourse.bass as bass
from concourse import bacc
import concourse.tile as tile
import concourse.mybir as mybir

F32 = mybir.dt.float32
BF16 = mybir.dt.bfloat16
F8 = mybir.dt.float8e4
NF8 = ml_dtypes.float8_e4m3
NBF = ml_dtypes.bfloat16
AF = mybir.ActivationFunctionType
DR = mybir.MatmulPerfMode.DoubleRow

H = 512
BL = 32          # batch per core
RING = 4


def build_gru(T=512, n_cores=8):
    nc = bacc.Bacc("TRN2", target_bir_lowering=False, debug=False,
                   num_devices=n_cores)
    xw_d = nc.dram_tensor("xw", (1, T * BL), BF16, kind="ExternalInput").ap()
    wn_d = nc.dram_tensor("wn", (128, 12 * H), BF16, kind="ExternalInput").ap()
    wrz_d = nc.dram_tensor("wrz8", (128, 24 * H), F8, kind="ExternalInput").ap()
    cA_d = nc.dram_tensor("cA", (3, H), BF16, kind="ExternalInput").ap()
    cB_d = nc.dram_tensor("cB", (3, H), BF16, kind="ExternalInput").ap()
    cC_d = nc.dram_tensor("cC", (5, H), BF16, kind="ExternalInput").ap()
    wfc_d = nc.dram_tensor("wfc", (128, 8), BF16, kind="ExternalInput").ap()
    fcb_d = nc.dram_tensor("fcb", (1, 2), BF16, kind="ExternalInput").ap()
    xsl_d = nc.dram_tensor("xsl", (5, 128), BF16, kind="ExternalInput").ap()
    out_d = nc.dram_tensor("out", (BL, 2), F32, kind="ExternalOutput").ap()

    with tile.TileContext(nc) as tc:
        import contextlib
        with contextlib.ExitStack() as ctx:
            const = ctx.enter_context(tc.tile_pool(name="const", bufs=1))
            state = ctx.enter_context(tc.tile_pool(name="state", bufs=1))
            scratch = ctx.enter_context(tc.tile_pool(name="scratch", bufs=2))
            psR = ctx.enter_context(tc.tile_pool(name="psR", bufs=2, space="PSUM"))
            psZ = ctx.enter_context(tc.tile_pool(name="psZ", bufs=2, space="PSUM"))
            psN = ctx.enter_context(tc.tile_pool(name="psN", bufs=2, space="PSUM"))
            psT = ctx.enter_context(tc.tile_pool(name="psT", bufs=1, space="PSUM"))
            psF = ctx.enter_context(tc.tile_pool(name="psF", bufs=1, space="PSUM"))

            # ---- persistent inputs ----
            xw = const.tile([1, T * BL], BF16)
            wn = const.tile([128, 12 * H], BF16)
            wrz = const.tile([128, 24 * H], F8)
            cA = const.tile([3, H], BF16)
            cB = const.tile([3, H], BF16)
            cC = const.tile([5, H], BF16)
            wfc = const.tile([128, 8], BF16)
            fcb = const.tile([1, 2], BF16)
            for t_, d_ in [(xw, xw_d), (wn, wn_d), (wrz, wrz_d), (cA, cA_d),
                           (cB, cB_d), (cC, cC_d), (wfc, wfc_d), (fcb, fcb_d)]:
                nc.sync.dma_start(out=t_[:], in_=d_)

            from concourse.masks import make_identity
            id2 = const.tile([64, 32], BF16)
            make_identity(nc, id2[0:32, :])
            make_identity(nc, id2[32:64, :])
            ones_t = const.tile([1, BL], BF16)
            nc.vector.memset(ones_t[:], 1.0)

            # xsel: [5,128] rows: x@0:32 | 1@0:32 | 1@32:64 | 1@64:96 | 1@96:128
            xsel = [state.tile([5, 128], BF16, name=f"xs{i}") for i in range(2)]
            for xs in xsel:
                nc.sync.dma_start(out=xs[:], in_=xsl_d)

            # h state ping-pong [64, 512] bf16 (rows 0:32 L0, 32:64 L1)
            h_sb = [state.tile([64, H], BF16, name=f"h{i}") for i in range(2)]
            # rings
            ra8 = [state.tile([128, 512], F8, name=f"ra8_{i}") for i in range(RING)]
            rb8 = [state.tile([128, 256], F8, name=f"rb8_{i}") for i in range(RING)]
            rb0 = [state.tile([128, 128], BF16, name=f"rb0_{i}") for i in range(RING)]
            rb1 = [state.tile([128, 128], BF16, name=f"rb1_{i}") for i in range(RING)]
            for t_ in h_sb + ra8 + rb8 + rb0 + rb1:
                nc.vector.memset(t_[:], 0.0)

            def dr_lhsT_a(ring_t, f):       # fused lhsT f=0..3: [128, 2, 64]
                return ring_t[:, 128 * f:128 * (f + 1)].rearrange(
                    "p (k m) -> p k m", k=2)

            def dr_lhsT_b(ring_t, fp):      # fused lhsT f=4,5 -> f'=0,1
                return ring_t[:, 128 * fp:128 * (fp + 1)].rearrange(
                    "p (k m) -> p k m", k=2)

            def dr_rhs(g, f):               # [128, 2, 512]
                o = ((g * 6 + f) * 2) * H
                return wrz[:, o:o + 2 * H].rearrange("p (k n) -> p k n", k=2)

            HH = H // 2

            def dr_rhs_h(g, f, hb):         # [128, 2, 256] n-half
                return dr_rhs(g, f)[:, :, hb * HH:(hb + 1) * HH]

            n_super = T + 2       # skew=2: L1 lags L0 by two steps
            pending = [None]        # deferred (hnew, w0, w1, t0, t1) of prev step

            def emit_rings_l1(hnew, tt1):
                """L1 transposes of prev step + rb1/rb8 ring writes."""
                trp = psT.tile([128, 512], BF16, tag="trp")
                trp1 = trp[:, 384:512]
                for c in range(4):
                    nc.tensor.transpose(
                        trp1[:, 32 * c:32 * (c + 1)],
                        hnew[32:64, 128 * c:128 * (c + 1)],
                        id2[32:64, :], tile_position=(32, 0))
                nc.scalar.activation(rb1[tt1 % RING][:], trp1[:, :], AF.Copy)
                nc.vector.tensor_copy(
                    out=rb8[tt1 % RING][:].rearrange("p (c m) -> p c m", c=4)[:, :, 32:64],
                    in_=trp1[:, :].rearrange("p (c m) -> p c m", c=4))
                return trp

            def emit_rings_l0(trp, hnew, tt0):
                """L0 transposes of prev step + ra8/rb0 ring writes.

                h0T(tt0) feeds: ra8[(tt0+1)%R] rows 0:64 (consumed next step,
                critical) and ra8[(tt0+2)%R] rows 64:128 (wi1 side, consumed
                in two steps — off the critical path)."""
                trpb = trp[:, 256:384]
                for c in range(4):
                    nc.tensor.transpose(
                        trpb[:, 32 * c:32 * (c + 1)],
                        hnew[0:32, 128 * c:128 * (c + 1)],
                        id2[0:32, :], tile_position=(0, 0))
                ra_f = ra8[(tt0 + 1) % RING]
                ra_o = ra8[(tt0 + 2) % RING]
                lo4 = lambda ap: ap.rearrange("p (f m) -> p f m", f=4)
                # fused-lhsT layout: col 128f + 64kt + 32sel + b
                nc.vector.tensor_copy(out=lo4(ra_f[0:64, :])[:, :, 0:32],
                                      in_=lo4(trpb[0:64, :]))
                nc.vector.tensor_copy(out=lo4(ra_f[0:64, :])[:, :, 64:96],
                                      in_=lo4(trpb[64:128, :]))
                nc.vector.tensor_copy(out=rb0[tt0 % RING][:], in_=trpb[:, :])
                nc.scalar.activation(lo4(ra_o[64:128, :])[:, :, 32:64],
                                     lo4(trpb[0:64, :]), AF.Copy)
                nc.scalar.activation(lo4(ra_o[64:128, :])[:, :, 96:128],
                                     lo4(trpb[64:128, :]), AF.Copy)

            for s in range(n_super):
                t0, t1 = s, s - 2
                l0 = s < T
                l1 = s >= 2
                par = s % 2
                lo = 0 if l0 else 32
                hi = 64 if l1 else 32

                pr = psR.tile([64, H], F32, tag="pr")
                pz = psZ.tile([64, H], F32, tag="pz")
                pn = psN.tile([128, H], F32, tag="pn")

                xs = xsel[par]
                if l0:
                    nc.scalar.activation(xs[0:1, 0:BL], xw[0:1, BL * t0:BL * (t0 + 1)],
                                         AF.Copy)

                ra_cur = ra8[s % RING]            # 0:64 h0T(s-1) | 64:128 h0T(s-2)
                rb8_prev2 = rb8[(t1 - 1) % RING]
                rb0_fresh = rb0[(t0 - 1) % RING]  # nh0 side
                rb0_old = rb0[t1 % RING]          # ni1 side (h0T(t1), 2 steps old)
                rb1_prev2 = rb1[(t1 - 1) % RING]

                # ---- combos (start accumulation in each bank) ----
                nc.tensor.matmul(pr[:, :], lhsT=xs[0:3, 0:64], rhs=cA[:],
                                 start=True, stop=False, tile_position=(0, 0),
                                 skip_group_check=True)
                nc.tensor.matmul(pz[:, :], lhsT=xs[0:3, 0:64], rhs=cB[:],
                                 start=True, stop=False, tile_position=(0, 0),
                                 skip_group_check=True)
                nc.tensor.matmul(pn[:, :], lhsT=xs[0:5, :], rhs=cC[:],
                                 start=True, stop=False, tile_position=(0, 0),
                                 skip_group_check=True)

                # ---- ni1 (2-step-old dep: pure filler work) ----
                for c in range(4):
                    nc.tensor.matmul(pn[32:64, :],
                                     lhsT=rb0_old[:, 32 * c:32 * (c + 1)],
                                     rhs=wn[:, (4 + c) * H:(5 + c) * H],
                                     start=False, stop=(c == 3),
                                     tile_position=(0, 32), skip_group_check=True)

                # ---- deferred L1 rings of prev step (feeds nh1/f45 below) ----
                trp_prev = None
                if pending[0] is not None and pending[0][2]:
                    trp_prev = emit_rings_l1(pending[0][0], pending[0][4])

                # ---- L1-side MMs: nh1, rz f=4,5 ----
                for c in range(4):
                    nc.tensor.matmul(pn[96:128, :],
                                     lhsT=rb1_prev2[:, 32 * c:32 * (c + 1)],
                                     rhs=wn[:, (8 + c) * H:(9 + c) * H],
                                     start=False, stop=(c == 3),
                                     tile_position=(0, 96), skip_group_check=True)
                for fp in range(2):
                    lh = dr_lhsT_b(rb8_prev2, fp)
                    nc.tensor.matmul(pr[:, :], lhsT=lh, rhs=dr_rhs(0, 4 + fp),
                                     start=False, stop=False, perf_mode=DR,
                                     tile_position=(0, 0), skip_group_check=True)
                    nc.tensor.matmul(pz[:, :], lhsT=lh, rhs=dr_rhs(1, 4 + fp),
                                     start=False, stop=False, perf_mode=DR,
                                     tile_position=(0, 0), skip_group_check=True)

                # ---- deferred L0 rings of prev step (feeds f0..3/nh0) ----
                if pending[0] is not None:
                    if pending[0][1]:
                        if trp_prev is None:
                            trp_prev = psT.tile([128, 512], BF16, tag="trp")
                        emit_rings_l0(trp_prev, pending[0][0], pending[0][3])
                    pending[0] = None

                # ---- fresh-dependency MMs: rz f=0..3, then nh0 ----
                for f in range(4):
                    lh = dr_lhsT_a(ra_cur, f)
                    nc.tensor.matmul(pr[:, :], lhsT=lh, rhs=dr_rhs(0, f),
                                     start=False, stop=(f == 3), perf_mode=DR,
                                     tile_position=(0, 0), skip_group_check=True)
                    nc.tensor.matmul(pz[:, :], lhsT=lh, rhs=dr_rhs(1, f),
                                     start=False, stop=(f == 3), perf_mode=DR,
                                     tile_position=(0, 0), skip_group_check=True)
                for c in range(4):
                    nc.tensor.matmul(pn[64:96, :],
                                     lhsT=rb0_fresh[:, 32 * c:32 * (c + 1)],
                                     rhs=wn[:, c * H:(c + 1) * H],
                                     start=False, stop=(c == 3),
                                     tile_position=(0, 64), skip_group_check=True)

                # ---- gate math ----
                rq = scratch.tile([64, H], BF16, tag="rq")
                zq = scratch.tile([64, H], BF16, tag="zq")
                oz = scratch.tile([64, H], BF16, tag="oz")
                tq = scratch.tile([64, H], BF16, tag="tq")
                uq = scratch.tile([64, H], BF16, tag="uq")
                nq = scratch.tile([64, H], BF16, tag="nq")
                zh = scratch.tile([64, H], BF16, tag="zh")
                pq = scratch.tile([64, H], BF16, tag="pq")
                hnew = h_sb[par]
                hold = h_sb[1 - par]

                for hb in range(2):
                    sl = slice(hb * HH, (hb + 1) * HH)
                    nc.scalar.activation(rq[lo:hi, sl], pr[lo:hi, sl], AF.Sigmoid)
                nc.scalar.activation(zq[lo:hi, :], pz[lo:hi, :], AF.Sigmoid)
                nc.scalar.activation(oz[lo:hi, :], pz[lo:hi, :], AF.Sigmoid,
                                     scale=-1.0)
                # zh = z * h_old (gpsimd, sbuf-only)
                nc.gpsimd.tensor_mul(out=zh[lo:hi, :], in0=zq[lo:hi, :],
                                     in1=hold[lo:hi, :])
                for hb in range(2):
                    sl = slice(hb * HH, (hb + 1) * HH)
                    nc.vector.tensor_mul(out=tq[lo:hi, sl], in0=rq[lo:hi, sl],
                                         in1=pn[64 + lo:64 + hi, sl])
                    nc.vector.tensor_add(out=uq[lo:hi, sl], in0=tq[lo:hi, sl],
                                         in1=pn[lo:hi, sl])
                    nc.scalar.activation(nq[lo:hi, sl], uq[lo:hi, sl], AF.Tanh)
                    nc.vector.tensor_mul(out=pq[lo:hi, sl], in0=oz[lo:hi, sl],
                                         in1=nq[lo:hi, sl])
                    nc.vector.tensor_add(out=hnew[lo:hi, sl], in0=pq[lo:hi, sl],
                                         in1=zh[lo:hi, sl])

                # ---- stash transposes + ring writes for next step's stream ----
                pending[0] = (hnew, l0, l1, t0, t1)

            if pending[0] is not None:
                trp_f = None
                if pending[0][2]:
                    trp_f = emit_rings_l1(pending[0][0], pending[0][4])
                if pending[0][1]:
                    if trp_f is None:
                        trp_f = psT.tile([128, 512], BF16, tag="trp")
                    emit_rings_l0(trp_f, pending[0][0], pending[0][3])
                pending[0] = None

            # ---- FC ----
            ps_fc = psF.tile([BL, 2], F32)
            hT_last = rb1[(T - 1) % RING]
            for c in range(4):
                nc.tensor.matmul(ps_fc[:, :], lhsT=hT_last[:, BL * c:BL * (c + 1)],
                                 rhs=wfc[:, 2 * c:2 * (c + 1)],
                                 start=(c == 0), stop=False, skip_group_check=True)
            nc.tensor.matmul(ps_fc[:, :], lhsT=ones_t[0:1, :], rhs=fcb[:],
                             start=False, stop=True, skip_group_check=True)
            out_sb = const.tile([BL, 2], F32)
            nc.vector.tensor_copy(out=out_sb[:], in_=ps_fc[:, :])
            nc.sync.dma_start(out=out_d, in_=out_sb[:])

    nc.compile()
    return nc


# ---------------- host-side packing ----------------

def pack_inputs(x, Wi0, bi0, Wi_rest, bi_rest, Wh, bh, fc_w, fc_b, n_cores=8):
    B, T = x.shape
    bl = B // n_cores
    assert bl == BL

    # n-gate weights, classic chunk layout: [wh0_n, wi1_n, wh1_n]
    wn = np.zeros((128, 12 * H), np.float32)
    for M, W in enumerate([Wh[0, 2], Wi_rest[0, 2], Wh[1, 2]]):
        for c in range(4):
            # wn[p, (4M+c)*H + n] = W[n, 128c+p]
            wn[:, (4 * M + c) * H:(4 * M + c + 1) * H] = W[:, 128 * c:128 * (c + 1)].T
    wn = wn.astype(NBF)

    # r/z fused fp8 rhs
    wrz = np.zeros((128, 24 * H), np.float32)
    for g in range(2):
        Wh0g, Wi1g, Wh1g = Wh[0, g], Wi_rest[0, g], Wh[1, g]
        for f in range(4):
            for kt in range(2):
                col = ((g * 6 + f) * 2 + kt) * H
                k0 = 128 * f + 64 * kt
                wrz[0:64, col:col + H] = Wh0g[:, k0:k0 + 64].T
                wrz[64:128, col:col + H] = Wi1g[:, k0:k0 + 64].T
        for fp in range(2):
            for kt in range(2):
                col = ((g * 6 + 4 + fp) * 2 + kt) * H
                k0 = 256 * fp + 128 * kt
                wrz[:, col:col + H] = Wh1g[:, k0:k0 + 128].T
    wrz = wrz.astype(NF8)

    # combo rhs
    cA = np.stack([Wi0[0, :, 0], bi0[0] + bh[0, 0], bi_rest[0, 0] + bh[1, 0]])
    cB = np.stack([Wi0[1, :, 0], bi0[1] + bh[0, 1], bi_rest[0, 1] + bh[1, 1]])
    cC = np.stack([Wi0[2, :, 0], bi0[2], bi_rest[0, 2], bh[0, 2], bh[1, 2]])
    cA = cA.astype(NBF); cB = cB.astype(NBF); cC = cC.astype(NBF)

    wfc = fc_w.T.reshape(4, 128, 2).transpose(1, 0, 2)
    wfc = np.ascontiguousarray(wfc).reshape(128, 8).astype(NBF)
    fcb = fc_b.reshape(1, 2).astype(NBF)

    xsl = np.zeros((5, 128), np.float32)
    for j in range(4):
        xsl[j + 1, 32 * j:32 * j + 32] = 1.0
    xsl = xsl.astype(NBF)

    in_maps = []
    for cix in range(n_cores):
        xc = x[cix * bl:(cix + 1) * bl, :]
        xw = xc.T.reshape(1, -1).astype(NBF)     # [1, T*32], t-major
        in_maps.append({
            "xw": xw, "wn": wn, "wrz8": wrz,
            "cA": cA, "cB": cB, "cC": cC, "wfc": wfc, "fcb": fcb, "xsl": xsl,
        })
    return in_maps


def unpack_outputs(results):
    return np.concatenate([r["out"] for r in results], axis=0)


# ---------------- public entry point ----------------
_CACHED = {}


def _get_nc(T):
    if T not in _CACHED:
        _CACHED[T] = build_gru(T=T)
    return _CACHED[T]


def kernel(x, Wi0, bi0, Wi_rest, bi_rest, Wh, bh, fc_w, fc_b):
    """Full-input 2-layer GRU (B=256, H=512) on 8 NeuronCores.

    Sharding: data-parallel over batch (32 per core), weights replicated.
    """
    from concourse.bass_utils import run_bass_kernel_spmd
    x = np.asarray(x); Wi0 = np.asarray(Wi0); bi0 = np.asarray(bi0)
    Wi_rest = np.asarray(Wi_rest); bi_rest = np.asarray(bi_rest)
    Wh = np.asarray(Wh); bh = np.asarray(bh)
    fc_w = np.asarray(fc_w); fc_b = np.asarray(fc_b)
    T = x.shape[1]
    nc = _get_nc(T)
    in_maps = pack_inputs(x, Wi0, bi0, Wi_rest, bi_rest, Wh, bh, fc_w, fc_b)
    res = run_bass_kernel_spmd(nc, in_maps, core_ids=list(range(8)))
    return unpack_outputs(res.results).astype(np.float32)
